# revision 6
# baseline (speedup 1.0000x reference)
"""Single-head causal self-attention on 8 Trainium2 NeuronCores.

Reference computation (per batch b):
    k = x @ Wk.T ; q = x @ Wq.T ; v = x @ Wv.T
    wei = softmax(mask(q @ k.T / sqrt(H)))
    out = wei @ v

Strategy (v3):
  - Data parallel: shard B=256 across 8 cores (32 batches each), replicate
    weights. No cross-core communication.
  - Host-side weight preprocessing (O(C^2), negligible): G = Wq.T @ Wk * scale
    and WvT = Wv.T are computed in numpy and shipped as inputs, so q @ k.T
    becomes x G x.T on chip and no weight transposes are needed.
  - x is shipped as bf16; x^T tiles are produced by the DMA XBAR transpose
    (2-byte path) straight from HBM, one instruction per PAIR of batches
    ([512, 384] -> [128, 3, 512]) -- zero tensor-engine transposes.
  - All matmuls run bf16 x bf16 -> fp32 PSUM. z2 = G^T x^T is computed per
    batch-pair with 512-wide moving operands.
  - Causal structure at 128-block granularity: the s-hi/t-lo score block is
    never computed; scores land transposed ST[s,t] packed [128, 384] per
    batch, one Exp activation, two gpsimd affine_selects mask the diagonal.
  - Softmax denominator: V is augmented with ones columns so the attention
    matmul also yields r[t]; normalization = reciprocal + per-partition mul.
  - Output is stored bf16 (halves store traffic); host upcasts to f32.
"""

import numpy as np
import ml_dtypes

import concourse.bass as bass
import concourse.mybir as mybir
from concourse import bacc
import concourse.tile as tile
from concourse.bass_utils import run_bass_kernel_spmd

B, T, C, H = 256, 256, 384, 384
NCORES = 8
NB = B // NCORES  # batches per core
P = 128
CC = C // P  # 3 chunks of the embedding dim
TC = T // P  # 2 chunks of the sequence dim
SCALE = float(H) ** -0.5
F32 = mybir.dt.float32
BF16 = mybir.dt.bfloat16
HP = H + 8  # v augmented with 8 ones columns (16B-aligned in bf16)
T2 = 2 * T  # 512: per-pair time span


def build_bass(nb: int = NB):
    assert nb % 2 == 0
    nc = bacc.Bacc(
        "TRN2",
        target_bir_lowering=False,
        debug=False,
        enable_asserts=False,
        num_devices=NCORES,
    )
    # x viewed flat as [(nb*T), C] so a batch pair is a 2D slice [512, C]
    x_d = nc.dram_tensor("x", [nb * T, C], BF16, kind="ExternalInput").ap()
    g_d = nc.dram_tensor("G", [C, C], BF16, kind="ExternalInput").ap()
    wvt_d = nc.dram_tensor("WvT", [C, H], BF16, kind="ExternalInput").ap()
    out_d = nc.dram_tensor("out", [nb * T, H], BF16, kind="ExternalOutput").ap()

    with tile.TileContext(nc) as tc:
        with (
            tc.tile_pool(name="const", bufs=1) as cpool,
            tc.tile_pool(name="sb", bufs=3) as sb,
            tc.tile_pool(name="ob", bufs=4) as obp,
            tc.tile_pool(name="ps", bufs=1, space="PSUM") as psp,
        ):
            # G tiles [c1 partition chunk, c2 free] and WvT tiles [c, h free]
            g_s, wvT_s = [], []
            for cc_ in range(CC):
                g_t = cpool.tile([P, C], BF16, name=f"g{cc_}")
                nc.sync.dma_start(g_t, g_d[cc_ * P : (cc_ + 1) * P, :])
                g_s.append(g_t)
                w_t = cpool.tile([P, H], BF16, name=f"wvT{cc_}")
                nc.sync.dma_start(w_t, wvt_d[cc_ * P : (cc_ + 1) * P, :])
                wvT_s.append(w_t)

            n_pairs = nb // 2
            xTp_tiles = {}

            def emit_transpose(p):
                if p >= n_pairs or p in xTp_tiles:
                    return
                # xTp[c, cc, u] = x[pair, u, cc*128+c], u in [0,512) spanning
                # both batches of the pair; one XBAR transpose per pair.
                xTp = sb.tile([P, CC, T2], BF16, name="xTp", tag="xTp")
                nc.sync.dma_start(
                    xTp, x_d[p * T2 : (p + 1) * T2, :], transpose=True
                )
                xTp_tiles[p] = xTp

            zt_tiles = {}

            def emit_z2(p):
                if p >= n_pairs:
                    return
                # z2 = G^T x^T for the pair: [C, 512] in 3 PSUM banks
                xTp = xTp_tiles[p]
                zt = sb.tile([P, CC, T2], BF16, name="zt", tag="zt")
                for c2 in range(CC):
                    pz = psp.tile([P, 512], F32, name=f"pz{c2}", tag=f"pz{c2}")
                    for c1 in range(CC):
                        nc.tensor.matmul(
                            pz,
                            lhsT=g_s[c1][:, c2 * P : (c2 + 1) * P],
                            rhs=xTp[:, c1, :],
                            start=(c1 == 0),
                            stop=(c1 == CC - 1),
                        )
                    nc.vector.tensor_copy(zt[:, c2, :], pz)
                zt_tiles[p] = zt

            emit_transpose(0)
            emit_transpose(1)
            emit_z2(0)

            for pr in range(n_pairs):
                xTp = xTp_tiles.pop(pr)
                zt = zt_tiles.pop(pr)
                for half in range(2):
                    off = half * T  # batch offset within the pair tiles
                    b = pr * 2 + half

                    # ST[s, t] packed [128, 384]: cols 0:256 = (s-lo, t full),
                    # cols 256:384 = (s-hi, t-hi). s-hi/t-lo never computed.
                    pst = psp.tile([P, 512], F32, name="pst", tag="pst")[
                        :, : T + P
                    ]
                    for cc_ in range(CC):
                        nc.tensor.matmul(
                            pst[:, 0:T],
                            lhsT=xTp[:, cc_, off : off + P],
                            rhs=zt[:, cc_, off : off + T],
                            start=(cc_ == 0),
                            stop=(cc_ == CC - 1),
                        )
                    for cc_ in range(CC):
                        nc.tensor.matmul(
                            pst[:, T : T + P],
                            lhsT=xTp[:, cc_, off + P : off + T],
                            rhs=zt[:, cc_, off + P : off + T],
                            start=(cc_ == 0),
                            stop=(cc_ == CC - 1),
                        )
                    et = sb.tile([P, T + P], BF16, name="et", tag="et")
                    nc.scalar.activation(
                        et, pst, mybir.ActivationFunctionType.Exp
                    )
                    # causal mask on the diagonal blocks: keep where t >= s
                    for col0 in (0, T):
                        nc.gpsimd.affine_select(
                            out=et[:, col0 : col0 + P],
                            in_=et[:, col0 : col0 + P],
                            compare_op=mybir.AluOpType.is_ge,
                            fill=0.0,
                            base=0,
                            channel_multiplier=-1,
                            pattern=[[1, P]],
                        )

                    # v_aug[sc] = [x[b] @ Wv.T | 1]  ([128, H+8] bf16)
                    # (emitted after ST so these matmuls hide the exp/affine
                    # latency before the attention matmuls need et)
                    vau = []
                    for sc in range(TC):
                        pv = psp.tile(
                            [P, 512], F32, name="pv", tag="pv", bufs=2
                        )[:, :H]
                        for cc_ in range(CC):
                            nc.tensor.matmul(
                                pv,
                                lhsT=xTp[
                                    :, cc_, off + sc * P : off + (sc + 1) * P
                                ],
                                rhs=wvT_s[cc_],
                                start=(cc_ == 0),
                                stop=(cc_ == CC - 1),
                            )
                        vt = sb.tile([P, HP], BF16, name=f"v{sc}", tag=f"v{sc}")
                        nc.vector.tensor_copy(vt[:, :H], pv)
                        nc.gpsimd.memset(vt[:, H:HP], 1.0)
                        vau.append(vt)

                    if half == 1:
                        # software pipeline: next pair's transpose + z2 run
                        # here so the tensor engine never waits on exp/affine
                        emit_transpose(pr + 2)
                        emit_z2(pr + 1)

                    # out[t, h] = (sum_s est[s, t] * v_aug[s, h]) / r[t]
                    for tcc in range(TC):
                        po = psp.tile(
                            [P, 512], F32, name="po", tag="po", bufs=2
                        )[:, :HP]
                        if tcc == 0:
                            nc.tensor.matmul(
                                po,
                                lhsT=et[:, 0:P],
                                rhs=vau[0],
                                start=True,
                                stop=True,
                            )
                        else:
                            nc.tensor.matmul(
                                po,
                                lhsT=et[:, P:T],
                                rhs=vau[0],
                                start=True,
                                stop=False,
                            )
                            nc.tensor.matmul(
                                po,
                                lhsT=et[:, T : T + P],
                                rhs=vau[1],
                                start=False,
                                stop=True,
                            )
                        rec = obp.tile([P, 1], F32, name="rec", tag="rec", bufs=8)
                        nc.vector.reciprocal(rec, po[:, H : H + 1])
                        ot = obp.tile([P, H], BF16, name="ot", tag="ot", bufs=8)
                        nc.scalar.mul(ot, po[:, :H], rec)
                        nc.sync.dma_start(
                            out_d[b * T + tcc * P : b * T + (tcc + 1) * P, :],
                            ot,
                        )

    nc.compile()
    return nc


_NC_CACHE = {}


def _get_nc(nb: int):
    if nb not in _NC_CACHE:
        _NC_CACHE[nb] = build_bass(nb)
    return _NC_CACHE[nb]


def prep_in_maps(x, Wk, Wq, Wv):
    """Host-side shard + weight-only preprocessing -> per-core input maps."""
    x = np.asarray(x, dtype=np.float32)
    Wk = np.asarray(Wk, dtype=np.float32)
    Wq = np.asarray(Wq, dtype=np.float32)
    Wv = np.asarray(Wv, dtype=np.float32)
    G = np.ascontiguousarray((Wq.T @ Wk) * SCALE).astype(ml_dtypes.bfloat16)
    WvT = np.ascontiguousarray(Wv.T).astype(ml_dtypes.bfloat16)
    nb = x.shape[0] // NCORES
    xb = np.ascontiguousarray(x).astype(ml_dtypes.bfloat16)
    xb = xb.reshape(NCORES, nb * x.shape[1], x.shape[2])
    return nb, [
        {"x": xb[i], "G": G, "WvT": WvT} for i in range(NCORES)
    ]


def kernel(x: np.ndarray, Wk: np.ndarray, Wq: np.ndarray, Wv: np.ndarray, **_):
    nb, in_maps = prep_in_maps(x, Wk, Wq, Wv)
    nc = _get_nc(nb)
    res = run_bass_kernel_spmd(nc, in_maps, core_ids=list(range(NCORES)))
    return np.concatenate(
        [
            np.asarray(r["out"]).astype(np.float32).reshape(nb, T, H)
            for r in res.results
        ],
        axis=0,
    )


if __name__ == "__main__":
    rng = np.random.default_rng(0)
    x = rng.standard_normal((B, T, C), dtype=np.float32)
    s = 1.0 / np.sqrt(C)
    Wk = rng.standard_normal((H, C), dtype=np.float32) * s
    Wq = rng.standard_normal((H, C), dtype=np.float32) * s
    Wv = rng.standard_normal((H, C), dtype=np.float32) * s
    out = kernel(x=x, Wk=Wk, Wq=Wq, Wv=Wv)
    print(out.shape, out.dtype)
